# revision 22
# baseline (speedup 1.0000x reference)
"""CNF vector-field + exact Jacobian-trace kernel for Trainium2 (8 NeuronCores).

Math: for each sample x (D=32), with inp = [x, t] (33,):
  h1 = tanh(inp @ W1 + b1); h2 = tanh(h1 @ W2 + b2); dx = h2 @ W3 + b3
  div = trace(J) = d1^T C d2,  C = W2 * (W3 @ W1r)^T,  d_i = 1 - h_i^2
  out = [dx, div]  (B, 33)

v2 implementation notes (vs the 33.6us baseline):
  - data-parallel over batch (2048 -> 8 x 256), weights replicated
  - host precomputes negmt = -(W3 @ W1r)^T (weight-only): kills the on-device
    W3-transpose + negMt matmul chain and 4 PSUM banks
  - P = -C = w2k * negmt chunk (DVE);  gt_m = sum_k P_k[:,m]^T [h1sq_k | 1]
    -> col 256 of each gt bank is vP chunk; g = raw - col; E = (h2sq-1)*g;
    div = (-1)^T E.  No vP row matmuls (saved 8 N=512/256 MMs)
  - w2 / negmt / h1t / pmat / h1sq travel in bf16 (W2 DMA halves; matmuls
    run 1 col/cycle regardless); x/W1/z1/h2t/dx/div path stays f32(r)
  - colp/rowp DMAs are gone (the [128,11] colp DMA alone took 2.5us to
    issue): bias columns ride in the x DMA, b3 row rides in x too,
    ones/-1 columns are GpSimd memsets
  - DMA queues: SP carries xaug then w2; ACT carries w1, negmt, w3; the
    issue serialisation that delayed w2's completion to 17.8us now has w2
    landing ~12.5us
  - 9 back-to-back spam matmuls on scratch SBUF warm the PE HAM clock-gate
    (4/8 -> 8/8) during the DMA wait, so the real matmul burst runs at
    2.4GHz instead of 1.2GHz (baseline flipped only at 22.5us of 29.5)
  - emission order = engine-stream order; z2 k-outer consumes h1t as tanh1
    lands, gt m-outer staggers PSUM-stop so the DVE t/e tail pipelines with
    the div matmuls
"""
import sys

for _p in ("/opt/trn_rl_repo", "/root/.axon_site/_ro/trn_rl_repo"):
    if _p not in sys.path:
        sys.path.append(_p)

import numpy as np
import ml_dtypes

B, D, H = 2048, 32, 512
NCORES = 8
BC = B // NCORES          # 256 rows per core
NK = H // 128             # 4 chunks of the hidden dim
XC = 112                  # xaug cols: 66 x | 4 bias1 | 4 b2 | 32 b3row | pad
W3C = 136                 # w3aug cols: 128 w3 | 1 b3col | pad

_CACHE = {}


def _build():
    import concourse.bass as bass
    import concourse.tile as tile
    from concourse import bacc, mybir
    from concourse.masks import make_identity

    f32 = mybir.dt.float32
    f32r = mybir.dt.float32r
    bf16 = mybir.dt.bfloat16
    AF = mybir.ActivationFunctionType
    ALU = mybir.AluOpType

    nc = bacc.Bacc("TRN2", target_bir_lowering=False, debug=False,
                   num_devices=NCORES)

    # xaug rows p: [x[p], x[128+p], bias1 (4), b2 (4), b3row (p==0), pad]
    x_ext = nc.dram_tensor("xa", [128, XC], f32r, kind="ExternalInput").ap()
    # w1 = [W1; b1] stacked then column-interleaved on host -> (16, 34, 32):
    # the interleave splits the DMA into 34*16 descriptors so all 16 HWDGE
    # queues stay busy (DMAs that leave queues empty get ~4us-late
    # completion semaphores)
    w1_ext = nc.dram_tensor("w1", [16, D + 2, 32], f32r, kind="ExternalInput").ap()
    w2_ext = nc.dram_tensor("w2", [H, H], bf16, kind="ExternalInput").ap()
    # negc = -(W2 * (W3 @ W1r)^T)  (H, H): P itself, host-precomputed
    negc_ext = nc.dram_tensor("negc", [H, H], bf16, kind="ExternalInput").ap()
    # w3aug[p, k*32+j] = W3[k*128+p, j]; col 128 = b3 (p<32)
    w3_ext = nc.dram_tensor("w3a", [128, W3C], f32r, kind="ExternalInput").ap()
    out_ext = nc.dram_tensor("out", [BC, D + 1], f32, kind="ExternalOutput").ap()

    with tile.TileContext(nc) as tc:
        with tc.tile_pool(name="const", bufs=1) as cpool, \
             tc.tile_pool(name="work", bufs=1) as wpool, \
             tc.tile_pool(name="ps", bufs=1, space="PSUM") as pps:

            def big_ps(nm):
                return pps.tile([128, 288], f32, name=nm, tag="big", bufs=6)

            def small_ps(nm, shape):
                return pps.tile(shape, f32, name=nm, tag="small", bufs=2)

            # -------- ACT spline-table preload (overlaps the DMA phase) ----
            dm0 = wpool.tile([1, 1], f32, name="dm0")
            dm1 = wpool.tile([1, 1], f32, name="dm1")
            nc.gpsimd.memset(dm0[:, :], 0.0)
            nc.scalar.activation(dm1[:, :], dm0[:, :], AF.Tanh)

            # ------------- input DMAs -------------
            # SP queue: w1 interleaved FIRST (the baseline-proven pattern
            # that completes ~10.1us), then xaug, then w2 (the big one)
            w1e = cpool.tile([D + 2, H], f32r, name="w1e")
            nc.sync.dma_start(
                out=w1e[:, :].rearrange("r (a b) -> r a b", a=16),
                in_=w1_ext.rearrange("a r b -> r a b"))

            xat = wpool.tile([128, XC], f32r, name="xat")
            nc.sync.dma_start(out=xat[:, :], in_=x_ext[:, :])

            w2all = cpool.tile([128, NK * H], bf16, name="w2all")
            nc.sync.dma_start(
                out=w2all[:, :].rearrange("p (k j) -> p k j", k=NK),
                in_=w2_ext.rearrange("(k p) j -> p k j", k=NK))
            w2k = [w2all[:, k * H:(k + 1) * H] for k in range(NK)]

            # ACT queue: negc (the P matrix, feeds gt), then w3aug
            pall = cpool.tile([128, NK * H], bf16, name="pall")
            nc.scalar.dma_start(
                out=pall[:, :].rearrange("p (k j) -> p k j", k=NK),
                in_=negc_ext.rearrange("(k p) j -> p k j", k=NK))
            pmat = [pall[:, k * H:(k + 1) * H] for k in range(NK)]

            w3a = cpool.tile([128, W3C], f32r, name="w3a")
            nc.scalar.dma_start(out=w3a[:, :], in_=w3_ext[:, :])
            w3k = [w3a[:, k * D:(k + 1) * D] for k in range(NK)]
            b3row = xat[0:1, 74:106]
            bias1c = [xat[:, 66 + m:67 + m] for m in range(NK)]
            b2c = [xat[:, 70 + m:71 + m] for m in range(NK)]

            # ------------- constants via memset (+DVE copy for f32r) -------
            scratch = wpool.tile([128, H], f32, name="scratch")
            nc.gpsimd.memset(scratch[:, :], 0.125)
            scratchr = wpool.tile([128, H], f32r, name="scratchr")
            nc.vector.tensor_copy(scratchr[:, :], scratch[:, :])
            negc0 = wpool.tile([128, 1], f32, name="negc0")
            nc.gpsimd.memset(negc0[:, :], -1.0)
            negcol = wpool.tile([128, 1], f32r, name="negcol")
            nc.vector.tensor_copy(negcol[:, :], negc0[:, :])
            ones0 = wpool.tile([1, BC], f32, name="ones0")
            nc.gpsimd.memset(ones0[:, :], 1.0)
            ones_row = wpool.tile([1, BC], f32r, name="ones_row")
            nc.vector.tensor_copy(ones_row[:, :], ones0[:, :])

            # h1sq tiles carry an appended ones column (col 256)
            h1sq = [wpool.tile([128, BC + 1], bf16, name=f"h1sq_{k}")
                    for k in range(NK)]
            for k in range(NK):
                nc.gpsimd.memset(h1sq[k][:, BC:BC + 1], 1.0)

            ident = cpool.tile([128, 128], f32, name="ident")
            make_identity(nc, ident[:, :])

            # -------- PE HAM warm-up: dependency-free spam matmuls --------
            # 5 f32r N=512 MMs (~2.1us) bridge the PE into the xT/z1/z2
            # stream; the combined continuous activity flips the HAM
            # clock-gate (4/8 -> 8/8) partway into the z2 burst
            spam_ps = small_ps("spam", [128, H])
            for _ in range(5):
                nc.tensor.matmul(spam_ps[:, :], scratchr[:, 0:128],
                                 scratchr[:, :], start=True, stop=True)

            # ---------------- x transpose: a0 = xs^T (32, 256) -------------
            a0 = wpool.tile([D, BC], f32r, name="a0")
            for i in range(2):
                xp = small_ps("xT", [D + 1, 128])
                nc.tensor.transpose(xp[:, :],
                                    xat[:, i * (D + 1):(i + 1) * (D + 1)].bitcast(f32),
                                    ident[:, :])
                nc.vector.tensor_copy(a0[:, i * 128:(i + 1) * 128], xp[0:D, :])

            # ---------------- layer 1: z1 -> tanh -> h1sq ----------------
            z1s = []
            for m in range(NK):
                z1 = big_ps("z1")
                nc.tensor.matmul(z1[:, 0:BC], w1e[0:D, m * 128:(m + 1) * 128],
                                 a0[:, :], start=True, stop=True)
                z1s.append(z1)
            h1t = []
            for m in range(NK):
                h = wpool.tile([128, BC], bf16, name=f"h1t_{m}")
                nc.scalar.activation(h[:, :], z1s[m][:, 0:BC], AF.Tanh,
                                     bias=bias1c[m].bitcast(f32))
                h1t.append(h)
            for m in range(NK):
                nc.vector.tensor_tensor(out=h1sq[m][:, 0:BC], in0=h1t[m][:, :],
                                        in1=h1t[m][:, :], op=ALU.mult)

            # ------- layer 2 + Jacobian chain, pipelined per m-chunk -------
            # z2 runs k-outer for k<3 (consuming h1t as tanh1 lands), then
            # per m: z2[m]'s last k-step stops the bank, tanh2[m] starts on
            # ACT while the PE immediately runs gt[m]'s 4 matmuls; the
            # DVE t/e chain and GpSimd h2sq follow per-m so the div inputs
            # stream out instead of bunching at the end.
            # gt_m = sum_k P_k[:,m]^T [h1sq_k | 1]; col BC = vP chunk;
            # g = raw - col; E = (h2sq-1)*g; div = (-1)^T E
            z2s = [big_ps("z2") for _ in range(NK)]
            for k in range(NK - 1):
                for m in range(NK):
                    nc.tensor.matmul(z2s[m][:, 0:BC],
                                     w2k[k][:, m * 128:(m + 1) * 128],
                                     h1t[k][:, :],
                                     start=(k == 0), stop=False)
            h2t, h2sq, vcol, tm, ee = [], [], [], [], []
            for m in range(NK):
                nc.tensor.matmul(z2s[m][:, 0:BC],
                                 w2k[NK - 1][:, m * 128:(m + 1) * 128],
                                 h1t[NK - 1][:, :], start=False, stop=True)
                h = wpool.tile([128, BC], f32r, name=f"h2t_{m}")
                nc.scalar.activation(h[:, :], z2s[m][:, 0:BC], AF.Tanh,
                                     bias=b2c[m].bitcast(f32))
                h2t.append(h)
                sq = wpool.tile([128, BC], f32, name=f"h2sq_{m}")
                nc.gpsimd.tensor_tensor(out=sq[:, :], in0=h[:, :].bitcast(f32),
                                        in1=h[:, :].bitcast(f32), op=ALU.mult)
                h2sq.append(sq)
                gt = big_ps("gt")
                for k in range(NK):
                    nc.tensor.matmul(gt[:, 0:BC + 1],
                                     pmat[k][:, m * 128:(m + 1) * 128],
                                     h1sq[k][:, :],
                                     start=(k == 0), stop=(k == NK - 1))
                vc = wpool.tile([128, 1], f32, name=f"vc_{m}")
                nc.scalar.activation(vc[:, :], gt[:, BC:BC + 1], AF.Copy)
                vcol.append(vc)
                t = wpool.tile([128, BC], f32r, name=f"t_{m}")
                nc.vector.tensor_scalar(out=t[:, :], in0=gt[:, 0:BC],
                                        scalar1=vc[:, :], scalar2=None,
                                        op0=ALU.subtract)
                tm.append(t)
                e = wpool.tile([128, BC], f32r, name=f"e_{m}")
                nc.vector.scalar_tensor_tensor(out=e[:, :], in0=sq[:, :],
                                               scalar=1.0, in1=t[:, :],
                                               op0=ALU.subtract, op1=ALU.mult)
                ee.append(e)

            # -------- dx = W3^T h2 + b3 ; div = (-1)^T E --------
            dx_ps = small_ps("dx_ps", [D, BC])
            for k in range(NK):
                nc.tensor.matmul(dx_ps[:, :], w3k[k], h2t[k][:, :],
                                 start=(k == 0), stop=False)
            nc.tensor.matmul(dx_ps[:, :], b3row, ones_row[:, :],
                             start=False, stop=True)
            outt = wpool.tile([D + 1, BC], f32, name="outt")
            nc.scalar.activation(outt[0:D, :], dx_ps[:, :], AF.Copy)
            div_ps = small_ps("div_ps", [1, BC])
            for m in range(NK):
                nc.tensor.matmul(div_ps[:, :], negcol[:, :], ee[m][:, :],
                                 start=(m == 0), stop=(m == NK - 1))
            nc.scalar.activation(outt[D:D + 1, :], div_ps[:, :], AF.Copy)

            # ------- transpose back to (256, 33) and store -------
            outs = wpool.tile([128, 2 * (D + 1)], f32, name="outs")
            for i in range(2):
                op = small_ps("outP", [128, D + 1])
                nc.tensor.transpose(op[:, :], outt[:, i * 128:(i + 1) * 128],
                                    ident[0:D + 1, 0:D + 1])
                nc.scalar.activation(outs[:, i * (D + 1):(i + 1) * (D + 1)],
                                     op[:, :], AF.Copy)
            nc.scalar.dma_start(
                out=out_ext.rearrange("(i p) c -> p i c", i=2),
                in_=outs[:, :].rearrange("p (i c) -> p i c", i=2))

    nc.compile()
    return nc


def _get_nc():
    if "nc" not in _CACHE:
        _CACHE["nc"] = _build()
    return _CACHE["nc"]


def _prep_inputs(t, x, W1, b1, W2, b2, W3, b3):
    t = np.asarray(t, dtype=np.float32)
    x = np.ascontiguousarray(np.asarray(x, dtype=np.float32))
    W1 = np.asarray(W1, dtype=np.float32)
    b1 = np.asarray(b1, dtype=np.float32)
    W2 = np.asarray(W2, dtype=np.float32)
    W3 = np.asarray(W3, dtype=np.float32)
    b3 = np.asarray(b3, dtype=np.float32)

    w1s = np.concatenate([W1, b1.reshape(1, H)], axis=0)
    w1s = np.ascontiguousarray(
        w1s.reshape(D + 2, 16, 32).transpose(1, 0, 2))  # (16, 34, 32)

    w2b = np.ascontiguousarray(W2.astype(ml_dtypes.bfloat16))

    Mt = (W3 @ W1[:D]).T.astype(np.float32)          # (H, H), Mt[a, j]
    negc = np.ascontiguousarray((-(W2 * Mt)).astype(ml_dtypes.bfloat16))

    w3a = np.zeros((128, W3C), dtype=np.float32)
    w3a[:, 0:128] = W3.reshape(NK, 128, D).transpose(1, 0, 2).reshape(128, 128)
    w3a[0:D, 128] = b3

    bias1 = (np.float32(t.ravel()[0]) * W1[D, :] + b1).astype(np.float32)
    b2a = np.asarray(b2, dtype=np.float32)

    xas = []
    for i in range(NCORES):
        xa = np.zeros((128, XC), dtype=np.float32)
        xc = x[i * BC:(i + 1) * BC]
        xa[:, 0:D + 1] = xc[0:128]
        xa[:, D + 1:2 * (D + 1)] = xc[128:256]
        xa[:, 66:70] = bias1.reshape(NK, 128).T
        xa[:, 70:74] = b2a.reshape(NK, 128).T
        xa[0, 74:106] = b3
        xas.append(xa)
    return xas, w1s, w2b, negc, w3a


def kernel(t, x, W1, b1, W2, b2, W3, b3):
    from concourse.bass_utils import run_bass_kernel_spmd

    nc = _get_nc()
    xas, w1s, w2b, negc, w3a = _prep_inputs(t, x, W1, b1, W2, b2, W3, b3)
    in_maps = []
    for i in range(NCORES):
        in_maps.append({
            "xa": xas[i], "w1": w1s, "w2": w2b, "negc": negc, "w3a": w3a,
        })
    res = run_bass_kernel_spmd(nc, in_maps, core_ids=list(range(NCORES)))
    return np.concatenate([res.results[i]["out"] for i in range(NCORES)], axis=0)


# revision 25
# speedup vs baseline: 1.1104x; 1.1104x over previous
"""CNF vector-field + exact Jacobian-trace kernel for Trainium2 (8 NeuronCores).

Math: for each sample x (D=32), with inp = [x, t] (33,):
  h1 = tanh(inp @ W1 + b1); h2 = tanh(h1 @ W2 + b2); dx = h2 @ W3 + b3
  div = trace(J) = d1^T C d2,  C = W2 * (W3 @ W1r)^T,  d_i = 1 - h_i^2
  out = [dx, div]  (B, 33)

v2 implementation notes (vs the 33.6us baseline):
  - data-parallel over batch (2048 -> 8 x 256), weights replicated
  - host precomputes negmt = -(W3 @ W1r)^T (weight-only): kills the on-device
    W3-transpose + negMt matmul chain and 4 PSUM banks
  - P = -C = w2k * negmt chunk (DVE);  gt_m = sum_k P_k[:,m]^T [h1sq_k | 1]
    -> col 256 of each gt bank is vP chunk; g = raw - col; E = (h2sq-1)*g;
    div = (-1)^T E.  No vP row matmuls (saved 8 N=512/256 MMs)
  - w2 / negmt / h1t / pmat / h1sq travel in bf16 (W2 DMA halves; matmuls
    run 1 col/cycle regardless); x/W1/z1/h2t/dx/div path stays f32(r)
  - colp/rowp DMAs are gone (the [128,11] colp DMA alone took 2.5us to
    issue): bias columns ride in the x DMA, b3 row rides in x too,
    ones/-1 columns are GpSimd memsets
  - DMA queues: SP carries xaug then w2; ACT carries w1, negmt, w3; the
    issue serialisation that delayed w2's completion to 17.8us now has w2
    landing ~12.5us
  - 9 back-to-back spam matmuls on scratch SBUF warm the PE HAM clock-gate
    (4/8 -> 8/8) during the DMA wait, so the real matmul burst runs at
    2.4GHz instead of 1.2GHz (baseline flipped only at 22.5us of 29.5)
  - emission order = engine-stream order; z2 k-outer consumes h1t as tanh1
    lands, gt m-outer staggers PSUM-stop so the DVE t/e tail pipelines with
    the div matmuls
"""
import sys

for _p in ("/opt/trn_rl_repo", "/root/.axon_site/_ro/trn_rl_repo"):
    if _p not in sys.path:
        sys.path.append(_p)

import numpy as np
import ml_dtypes

B, D, H = 2048, 32, 512
NCORES = 8
BC = B // NCORES          # 256 rows per core
NK = H // 128             # 4 chunks of the hidden dim
XC = 112                  # xaug cols: 66 x | 4 bias1 | 4 b2 | 32 b3row | pad
W3C = 136                 # w3aug cols: 128 w3 | 1 b3col | pad

_CACHE = {}


def _build():
    import concourse.bass as bass
    import concourse.tile as tile
    from concourse import bacc, mybir
    from concourse.masks import make_identity

    f32 = mybir.dt.float32
    f32r = mybir.dt.float32r
    bf16 = mybir.dt.bfloat16
    AF = mybir.ActivationFunctionType
    ALU = mybir.AluOpType

    nc = bacc.Bacc("TRN2", target_bir_lowering=False, debug=False,
                   num_devices=NCORES)

    # xaug rows p: [x[p], x[128+p], bias1 (4), b2 (4), b3row (p==0), pad]
    x_ext = nc.dram_tensor("xa", [128, XC], f32r, kind="ExternalInput").ap()
    # w1 = [W1; b1] stacked (34, 512)
    w1_ext = nc.dram_tensor("w1", [D + 2, H], f32r, kind="ExternalInput").ap()
    w2_ext = nc.dram_tensor("w2", [H, H], bf16, kind="ExternalInput").ap()
    # negc = -(W2 * (W3 @ W1r)^T)  (H, H): P itself, host-precomputed
    negc_ext = nc.dram_tensor("negc", [H, H], bf16, kind="ExternalInput").ap()
    # w3aug[p, k*32+j] = W3[k*128+p, j]; col 128 = b3 (p<32)
    w3_ext = nc.dram_tensor("w3a", [128, W3C], f32r, kind="ExternalInput").ap()
    out_ext = nc.dram_tensor("out", [BC, D + 1], f32, kind="ExternalOutput").ap()

    with tile.TileContext(nc) as tc:
        with tc.tile_pool(name="const", bufs=1) as cpool, \
             tc.tile_pool(name="work", bufs=1) as wpool, \
             tc.tile_pool(name="ps", bufs=1, space="PSUM") as pps:

            def big_ps(nm):
                return pps.tile([128, 288], f32, name=nm, tag="big", bufs=6)

            def small_ps(nm, shape):
                return pps.tile(shape, f32, name=nm, tag="small", bufs=2)

            # -------- ACT spline-table preload (overlaps the DMA phase) ----
            dm0 = wpool.tile([1, 1], f32, name="dm0")
            dm1 = wpool.tile([1, 1], f32, name="dm1")
            nc.gpsimd.memset(dm0[:, :], 0.0)
            nc.scalar.activation(dm1[:, :], dm0[:, :], AF.Tanh)

            # ------------- input DMAs -------------
            # SP queue: xaug and w1 first (they gate the z1 chain),
            # then w2 (the big one)
            xat = wpool.tile([128, XC], f32r, name="xat")
            nc.sync.dma_start(out=xat[:, :], in_=x_ext[:, :])

            w1e = cpool.tile([D + 2, H], f32r, name="w1e")
            nc.sync.dma_start(out=w1e[:, :], in_=w1_ext[:, :])

            w2all = cpool.tile([128, NK * H], bf16, name="w2all")
            nc.sync.dma_start(
                out=w2all[:, :].rearrange("p (k j) -> p k j", k=NK),
                in_=w2_ext.rearrange("(k p) j -> p k j", k=NK))
            w2k = [w2all[:, k * H:(k + 1) * H] for k in range(NK)]

            # ACT queue: negc (the P matrix, feeds gt), then w3aug
            pall = cpool.tile([128, NK * H], bf16, name="pall")
            nc.scalar.dma_start(
                out=pall[:, :].rearrange("p (k j) -> p k j", k=NK),
                in_=negc_ext.rearrange("(k p) j -> p k j", k=NK))
            pmat = [pall[:, k * H:(k + 1) * H] for k in range(NK)]

            w3a = cpool.tile([128, W3C], f32r, name="w3a")
            nc.scalar.dma_start(out=w3a[:, :], in_=w3_ext[:, :])
            w3k = [w3a[:, k * D:(k + 1) * D] for k in range(NK)]
            b3row = xat[0:1, 74:106]
            bias1c = [xat[:, 66 + m:67 + m] for m in range(NK)]
            b2c = [xat[:, 70 + m:71 + m] for m in range(NK)]

            # ------------- constants via memset (+DVE copy for f32r) -------
            scratch = wpool.tile([128, H], f32, name="scratch")
            nc.gpsimd.memset(scratch[:, :], 0.125)
            scratchr = wpool.tile([128, H], f32r, name="scratchr")
            nc.vector.tensor_copy(scratchr[:, :], scratch[:, :])
            negc0 = wpool.tile([128, 1], f32, name="negc0")
            nc.gpsimd.memset(negc0[:, :], -1.0)
            negcol = wpool.tile([128, 1], f32r, name="negcol")
            nc.vector.tensor_copy(negcol[:, :], negc0[:, :])
            ones0 = wpool.tile([1, BC], f32, name="ones0")
            nc.gpsimd.memset(ones0[:, :], 1.0)
            ones_row = wpool.tile([1, BC], f32r, name="ones_row")
            nc.vector.tensor_copy(ones_row[:, :], ones0[:, :])

            # h1sq tiles carry an appended ones column (col 256)
            h1sq = [wpool.tile([128, BC + 1], bf16, name=f"h1sq_{k}")
                    for k in range(NK)]
            for k in range(NK):
                nc.gpsimd.memset(h1sq[k][:, BC:BC + 1], 1.0)

            ident = cpool.tile([128, 128], f32, name="ident")
            make_identity(nc, ident[:, :])

            # -------- PE HAM warm-up: dependency-free spam matmuls --------
            # 5 f32r N=512 MMs (~2.1us) bridge the PE into the xT/z1/z2
            # stream; the combined continuous activity flips the HAM
            # clock-gate (4/8 -> 8/8) partway into the z2 burst
            spam_ps = small_ps("spam", [128, H])
            for _ in range(5):
                nc.tensor.matmul(spam_ps[:, :], scratchr[:, 0:128],
                                 scratchr[:, :], start=True, stop=True)

            # ---------------- x transpose: a0 = xs^T (32, 256) -------------
            a0 = wpool.tile([D, BC], f32r, name="a0")
            for i in range(2):
                xp = small_ps("xT", [D + 1, 128])
                nc.tensor.transpose(xp[:, :],
                                    xat[:, i * (D + 1):(i + 1) * (D + 1)].bitcast(f32),
                                    ident[:, :])
                nc.vector.tensor_copy(a0[:, i * 128:(i + 1) * 128], xp[0:D, :])

            # ---------------- layer 1: z1 -> tanh -> h1sq ----------------
            z1s = []
            for m in range(NK):
                z1 = big_ps("z1")
                nc.tensor.matmul(z1[:, 0:BC], w1e[0:D, m * 128:(m + 1) * 128],
                                 a0[:, :], start=True, stop=True)
                z1s.append(z1)
            h1t = []
            for m in range(NK):
                h = wpool.tile([128, BC], bf16, name=f"h1t_{m}")
                nc.scalar.activation(h[:, :], z1s[m][:, 0:BC], AF.Tanh,
                                     bias=bias1c[m].bitcast(f32))
                h1t.append(h)
            for m in range(NK):
                nc.vector.tensor_tensor(out=h1sq[m][:, 0:BC], in0=h1t[m][:, :],
                                        in1=h1t[m][:, :], op=ALU.mult)

            # ------- layer 2 + Jacobian chain, pipelined per m-chunk -------
            # z2 runs k-outer for k<3 (consuming h1t as tanh1 lands), then
            # per m: z2[m]'s last k-step stops the bank, tanh2[m] starts on
            # ACT while the PE immediately runs gt[m]'s 4 matmuls; the
            # DVE t/e chain and GpSimd h2sq follow per-m so the div inputs
            # stream out instead of bunching at the end.
            # gt_m = sum_k P_k[:,m]^T [h1sq_k | 1]; col BC = vP chunk;
            # g = raw - col; E = (h2sq-1)*g; div = (-1)^T E
            z2s = [big_ps("z2") for _ in range(NK)]
            for k in range(NK - 1):
                for m in range(NK):
                    nc.tensor.matmul(z2s[m][:, 0:BC],
                                     w2k[k][:, m * 128:(m + 1) * 128],
                                     h1t[k][:, :],
                                     start=(k == 0), stop=False)
            h2t, h2sq, vcol, tm, ee = [], [], [], [], []
            for m in range(NK):
                nc.tensor.matmul(z2s[m][:, 0:BC],
                                 w2k[NK - 1][:, m * 128:(m + 1) * 128],
                                 h1t[NK - 1][:, :], start=False, stop=True)
                h = wpool.tile([128, BC], f32r, name=f"h2t_{m}")
                nc.scalar.activation(h[:, :], z2s[m][:, 0:BC], AF.Tanh,
                                     bias=b2c[m].bitcast(f32))
                h2t.append(h)
                sq = wpool.tile([128, BC], f32, name=f"h2sq_{m}")
                nc.gpsimd.tensor_tensor(out=sq[:, :], in0=h[:, :].bitcast(f32),
                                        in1=h[:, :].bitcast(f32), op=ALU.mult)
                h2sq.append(sq)
                gt = big_ps("gt")
                for k in range(NK):
                    nc.tensor.matmul(gt[:, 0:BC + 1],
                                     pmat[k][:, m * 128:(m + 1) * 128],
                                     h1sq[k][:, :],
                                     start=(k == 0), stop=(k == NK - 1))
                vc = wpool.tile([128, 1], f32, name=f"vc_{m}")
                nc.scalar.activation(vc[:, :], gt[:, BC:BC + 1], AF.Copy)
                vcol.append(vc)
                t = wpool.tile([128, BC], f32r, name=f"t_{m}")
                nc.vector.tensor_scalar(out=t[:, :], in0=gt[:, 0:BC],
                                        scalar1=vc[:, :], scalar2=None,
                                        op0=ALU.subtract)
                tm.append(t)
                e = wpool.tile([128, BC], f32r, name=f"e_{m}")
                nc.vector.scalar_tensor_tensor(out=e[:, :], in0=sq[:, :],
                                               scalar=1.0, in1=t[:, :],
                                               op0=ALU.subtract, op1=ALU.mult)
                ee.append(e)

            # -------- dx = W3^T h2 + b3 ; div = (-1)^T E --------
            dx_ps = small_ps("dx_ps", [D, BC])
            for k in range(NK):
                nc.tensor.matmul(dx_ps[:, :], w3k[k], h2t[k][:, :],
                                 start=(k == 0), stop=False)
            nc.tensor.matmul(dx_ps[:, :], b3row, ones_row[:, :],
                             start=False, stop=True)
            outt = wpool.tile([D + 1, BC], f32, name="outt")
            nc.scalar.activation(outt[0:D, :], dx_ps[:, :], AF.Copy)
            div_ps = small_ps("div_ps", [1, BC])
            for m in range(NK):
                nc.tensor.matmul(div_ps[:, :], negcol[:, :], ee[m][:, :],
                                 start=(m == 0), stop=(m == NK - 1))
            nc.scalar.activation(outt[D:D + 1, :], div_ps[:, :], AF.Copy)

            # ------- transpose back to (256, 33) and store -------
            outs = wpool.tile([128, 2 * (D + 1)], f32, name="outs")
            for i in range(2):
                op = small_ps("outP", [128, D + 1])
                nc.tensor.transpose(op[:, :], outt[:, i * 128:(i + 1) * 128],
                                    ident[0:D + 1, 0:D + 1])
                nc.scalar.activation(outs[:, i * (D + 1):(i + 1) * (D + 1)],
                                     op[:, :], AF.Copy)
            nc.scalar.dma_start(
                out=out_ext.rearrange("(i p) c -> p i c", i=2),
                in_=outs[:, :].rearrange("p (i c) -> p i c", i=2))

    nc.compile()
    return nc


def _get_nc():
    if "nc" not in _CACHE:
        _CACHE["nc"] = _build()
    return _CACHE["nc"]


def _prep_inputs(t, x, W1, b1, W2, b2, W3, b3):
    t = np.asarray(t, dtype=np.float32)
    x = np.ascontiguousarray(np.asarray(x, dtype=np.float32))
    W1 = np.asarray(W1, dtype=np.float32)
    b1 = np.asarray(b1, dtype=np.float32)
    W2 = np.asarray(W2, dtype=np.float32)
    W3 = np.asarray(W3, dtype=np.float32)
    b3 = np.asarray(b3, dtype=np.float32)

    w1s = np.ascontiguousarray(
        np.concatenate([W1, b1.reshape(1, H)], axis=0))  # (34, 512)

    w2b = np.ascontiguousarray(W2.astype(ml_dtypes.bfloat16))

    Mt = (W3 @ W1[:D]).T.astype(np.float32)          # (H, H), Mt[a, j]
    negc = np.ascontiguousarray((-(W2 * Mt)).astype(ml_dtypes.bfloat16))

    w3a = np.zeros((128, W3C), dtype=np.float32)
    w3a[:, 0:128] = W3.reshape(NK, 128, D).transpose(1, 0, 2).reshape(128, 128)
    w3a[0:D, 128] = b3

    bias1 = (np.float32(t.ravel()[0]) * W1[D, :] + b1).astype(np.float32)
    b2a = np.asarray(b2, dtype=np.float32)

    xas = []
    for i in range(NCORES):
        xa = np.zeros((128, XC), dtype=np.float32)
        xc = x[i * BC:(i + 1) * BC]
        xa[:, 0:D + 1] = xc[0:128]
        xa[:, D + 1:2 * (D + 1)] = xc[128:256]
        xa[:, 66:70] = bias1.reshape(NK, 128).T
        xa[:, 70:74] = b2a.reshape(NK, 128).T
        xa[0, 74:106] = b3
        xas.append(xa)
    return xas, w1s, w2b, negc, w3a


def kernel(t, x, W1, b1, W2, b2, W3, b3):
    from concourse.bass_utils import run_bass_kernel_spmd

    nc = _get_nc()
    xas, w1s, w2b, negc, w3a = _prep_inputs(t, x, W1, b1, W2, b2, W3, b3)
    in_maps = []
    for i in range(NCORES):
        in_maps.append({
            "xa": xas[i], "w1": w1s, "w2": w2b, "negc": negc, "w3a": w3a,
        })
    res = run_bass_kernel_spmd(nc, in_maps, core_ids=list(range(NCORES)))
    return np.concatenate([res.results[i]["out"] for i in range(NCORES)], axis=0)
